# revision 17
# baseline (speedup 1.0000x reference)
"""Causal self-attention (single head) on 8 Trainium2 NeuronCores.

Sharding: 8 cores = 4 batches x 2 query-tile parity sets. Core c handles
batch (c % 4). Cores 0-3 take query tiles t in {15,13,...,1} (128 rows
each), cores 4-7 take t in {14,12,...,0}. Attention iteration i=0..7 uses
a fixed causal extent E(i) = 16-2i k-tiles, so a single SPMD program
serves all cores; even-parity cores waste one fully-masked k-tile per
iteration.

Host passes x.T (plus the core's own query columns pre-gathered) and W.T
per core so the device never transposes inputs; operands are fp16 with
f32 PSUM accumulation. Softmax skips max-subtraction (scores/32 stay in a
safe exp range) and gets row sums free via the activation accum_out. All
operands stay SBUF-resident; Q.T is produced straight into SBUF.
"""

import sys

for _p in ("/opt/trn_rl_repo", "/root/.axon_site/_ro/trn_rl_repo"):
    if _p not in sys.path:
        sys.path.append(_p)

import numpy as np

import concourse.bass as bass  # noqa: F401
import concourse.mybir as mybir
import concourse.tile as tile
from concourse import bacc
from concourse.bass_utils import run_bass_kernel_spmd

F32 = mybir.dt.float32
F16 = mybir.dt.float16

BATCH, SEQ, D, P = 4, 2048, 1024, 1024
N_CORES = 8
QT = 128          # query tile rows
KTL = 128         # key tile
NBLK = 512        # matmul moving free dim
ND = D // 128     # 8 d-tiles
NP = P // 128     # 8 p-tiles
NKT = SEQ // KTL  # 16 k-tiles
NQT = 8           # q-tiles per core
SCALE = 1.0 / float(np.sqrt(P))
NEG = -1e9


def _extent(i):
    return 16 - 2 * i


def _chunks(width):
    out = []
    w = width
    while w >= NBLK:
        out.append(NBLK)
        w -= NBLK
    if w:
        assert w == 256, w
        out.append(256)
    return out


def build_program():
    nc = bacc.Bacc("TRN2", target_bir_lowering=False)

    xT = nc.dram_tensor("xT", [D, SEQ], F16, kind="ExternalInput")
    xq_in = nc.dram_tensor("xqcols", [D, NQT * QT], F16, kind="ExternalInput")
    WqT = nc.dram_tensor("WqT", [D, P], F16, kind="ExternalInput")
    WkT = nc.dram_tensor("WkT", [D, P], F16, kind="ExternalInput")
    WvT = nc.dram_tensor("WvT", [D, P], F16, kind="ExternalInput")
    mask = nc.dram_tensor("mask", [QT, 256], F32, kind="ExternalInput")
    ident_in = nc.dram_tensor("ident", [128, 128], F16, kind="ExternalInput")
    out = nc.dram_tensor("out", [NQT * QT, P], F32, kind="ExternalOutput")

    # [128, dt, cols] views (partition dim first); full-row reads keep the
    # DMA's contiguous runs at row length (2-4KB), not a sliced 1KB.
    xT_r = xT.rearrange("(dt dp) s -> dp dt s", dp=128)
    xq_r = xq_in.rearrange("(dt dp) q -> dp dt q", dp=128)
    wq_r = WqT.rearrange("(dt dp) p -> dp dt p", dp=128)
    wk_r = WkT.rearrange("(dt dp) p -> dp dt p", dp=128)
    wv_r = WvT.rearrange("(dt dp) p -> dp dt p", dp=128)

    with tile.TileContext(nc) as tc:
        with (
            tc.tile_pool(name="resident", bufs=1) as resident,
            tc.tile_pool(name="wrow", bufs=2) as wrow,
            tc.tile_pool(name="small", bufs=6) as small,
            tc.tile_pool(name="outp", bufs=2) as outp,
            tc.tile_pool(name="xqp", bufs=1) as xqp,
            tc.tile_pool(name="p0psum", bufs=2, space="PSUM") as p0psum,
            tc.tile_pool(name="spsum", bufs=2, space="PSUM") as spsum,
            tc.tile_pool(name="zpsum", bufs=2, space="PSUM") as zpsum,
            tc.tile_pool(name="tpsum", bufs=2, space="PSUM") as tpsum,
        ):
            kt_sb = resident.tile([128, NP, SEQ], F16)    # K.T  [p, k]
            v_sb = resident.tile([128, NKT, P], F16)      # V    [k, p]
            qt_all = resident.tile([128, NP, NQT * QT], F16)  # Q.T [p, q]
            xk_all = resident.tile([128, ND, SEQ], F16)   # x.T resident
            wq_sb = resident.tile([128, ND, P], F16)
            wk_sb = resident.tile([128, ND, P], F16)
            wv_sb = resident.tile([128, ND, P], F16)
            mask_sb = resident.tile([QT, 256], F32)
            ident = resident.tile([128, 128], F16)

            # startup loads: small constants, then per-d wq/xq pieces on the
            # sync queue (compute starts after the first pieces); bulk
            # tensors on the scalar queue, needed only ~40us in.
            for d in range(ND):
                nc.sync.dma_start(
                    out=xk_all[:, d, :], in_=xT_r[:, d, :])
                if d < 2:
                    nc.sync.dma_start(out=wk_sb[:, d, :], in_=wk_r[:, d, :])
            for d in range(2, ND):
                nc.sync.dma_start(out=wk_sb[:, d, :], in_=wk_r[:, d, :])
            nc.scalar.dma_start(out=wv_sb, in_=wv_r)
            nc.scalar.dma_start(out=wq_sb, in_=wq_r)
            xq0 = xqp.tile([128, ND, NBLK], F16, tag="xq")
            nc.scalar.dma_start(out=xq0, in_=xq_r[:, :, 0:NBLK])
            nc.scalar.dma_start(out=mask_sb, in_=mask[:, :])
            nc.scalar.dma_start(out=ident, in_=ident_in[:, :])

            # --- K.T and V production; Q.T produced between halves ---
            for kb in range(SEQ // NBLK):
                if kb == 2:
                    for qg in range(2):
                        if qg == 0:
                            xq = xq0
                        else:
                            xq = xqp.tile([128, ND, NBLK], F16, tag="xq")
                            nc.sync.dma_start(
                                out=xq,
                                in_=xq_r[:, :, qg * NBLK:(qg + 1) * NBLK])
                        for pt in range(NP):
                            ps = p0psum.tile([128, NBLK], F32, tag="p0")
                            for d in range(ND):
                                nc.tensor.matmul(
                                    ps,
                                    wq_sb[:, d, pt * 128:(pt + 1) * 128],
                                    xq[:, d, :],
                                    start=(d == 0),
                                    stop=(d == ND - 1),
                                )
                            nc.scalar.copy(
                                qt_all[:, pt, qg * NBLK:(qg + 1) * NBLK], ps)
                for pt in range(NP):
                    ps = p0psum.tile([128, NBLK], F32, tag="p0")
                    for d in range(ND):
                        nc.tensor.matmul(
                            ps,
                            wk_sb[:, d, pt * 128:(pt + 1) * 128],
                            xk_all[:, d, kb * NBLK:(kb + 1) * NBLK],
                            start=(d == 0),
                            stop=(d == ND - 1),
                        )
                    nc.scalar.copy(kt_sb[:, pt, kb * NBLK:(kb + 1) * NBLK], ps)
                for j in range(NBLK // KTL):
                    ktile = kb * (NBLK // KTL) + j
                    for pb in range(2):
                        ps = p0psum.tile([128, NBLK], F32, tag="p0")
                        for d in range(ND):
                            nc.tensor.matmul(
                                ps,
                                xk_all[:, d, ktile * 128:(ktile + 1) * 128],
                                wv_sb[:, d, pb * NBLK:(pb + 1) * NBLK],
                                start=(d == 0),
                                stop=(d == ND - 1),
                            )
                        nc.vector.tensor_copy(
                            v_sb[:, ktile, pb * NBLK:(pb + 1) * NBLK], ps)

            # --- attention, smallest extent first (unlocks earliest) ---
            for i in (7, 6, 5, 4, 3, 2, 1, 0):
                ext = _extent(i)
                width = ext * KTL
                chunks = _chunks(width)

                s_ps = []
                off = 0
                for cw in chunks:
                    ps_full = spsum.tile([QT, NBLK], F32, tag="s")
                    ps = ps_full[:, :cw]
                    for pt in range(NP):
                        nc.tensor.matmul(
                            ps,
                            qt_all[:, pt, i * QT:(i + 1) * QT],
                            kt_sb[:, pt, off:off + cw],
                            start=(pt == 0),
                            stop=(pt == NP - 1),
                        )
                    s_ps.append((ps, off, cw))
                    off += cw

                # additive causal mask on the last 256 columns of the row
                last_ps, _, last_w = s_ps[-1]
                nc.vector.tensor_add(
                    last_ps[:, last_w - 256:last_w],
                    last_ps[:, last_w - 256:last_w],
                    mask_sb,
                )

                # exp((s + m) * scale) -> fp16 weights row; row sums free
                w_sb = wrow.tile([QT, width], F16, tag="w")
                lparts = small.tile([QT, len(chunks)], F32, tag="lp")
                for ci, (ps, off_c, cw) in enumerate(s_ps):
                    nc.scalar.activation(
                        w_sb[:, off_c:off_c + cw],
                        ps,
                        mybir.ActivationFunctionType.Exp,
                        scale=SCALE,
                        accum_out=lparts[:, ci:ci + 1],
                    )

                lsum = small.tile([QT, 1], F32, tag="ls")
                nc.vector.reduce_sum(lsum, lparts, axis=mybir.AxisListType.X)
                rl = small.tile([QT, 1], F32, tag="rl")
                nc.vector.reciprocal(rl, lsum)

                # AV: transpose each weight block on PE, accumulate Z
                z0 = zpsum.tile([QT, NBLK], F32, tag="z")
                z1 = zpsum.tile([QT, NBLK], F32, tag="z")
                for kt in range(ext):
                    tp = tpsum.tile([128, 128], F16, tag="tp")
                    nc.tensor.transpose(
                        tp, w_sb[:, kt * 128:(kt + 1) * 128], ident)
                    wT = small.tile([128, 128], F16, tag="wT")
                    nc.vector.tensor_copy(wT, tp)
                    nc.tensor.matmul(
                        z0, wT, v_sb[:, kt, 0:NBLK],
                        start=(kt == 0), stop=(kt == ext - 1),
                    )
                    nc.tensor.matmul(
                        z1, wT, v_sb[:, kt, NBLK:P],
                        start=(kt == 0), stop=(kt == ext - 1),
                    )

                o_sb = outp.tile([QT, P], F32, tag="o")
                nc.vector.tensor_scalar_mul(o_sb[:, 0:NBLK], z0, rl)
                nc.vector.tensor_scalar_mul(o_sb[:, NBLK:P], z1, rl)
                nc.sync.dma_start(out=out[i * QT:(i + 1) * QT, :], in_=o_sb)

    nc.compile()
    return nc


def _tiles_for_core(c):
    """Global 128-row query-tile indices, in program order i=0..7."""
    return [(15 - 2 * i) if c < 4 else (14 - 2 * i) for i in range(NQT)]


def _host_prep(inputs, Wq, Wk, Wv):
    x = np.asarray(inputs, dtype=np.float32)
    WqT = np.ascontiguousarray(
        np.asarray(Wq, dtype=np.float32).T.astype(np.float16))
    WkT = np.ascontiguousarray(
        np.asarray(Wk, dtype=np.float32).T.astype(np.float16))
    WvT = np.ascontiguousarray(
        np.asarray(Wv, dtype=np.float32).T.astype(np.float16))

    qi = np.arange(QT)[:, None]
    ki = np.arange(128)[None, :]
    tri = np.where(qi >= ki, 0.0, NEG).astype(np.float32)
    mask_hi = np.concatenate([np.zeros((QT, 128), np.float32), tri], axis=1)
    mask_lo = np.concatenate(
        [tri, np.full((QT, 128), NEG, np.float32)], axis=1)

    in_maps = []
    xT_cache = {}
    for c in range(N_CORES):
        b = c % 4
        if b not in xT_cache:
            xT_cache[b] = np.ascontiguousarray(x[b].T.astype(np.float16))
        xTb = xT_cache[b]
        cols = np.concatenate(
            [xTb[:, t * QT:(t + 1) * QT] for t in _tiles_for_core(c)], axis=1)
        in_maps.append({
            "xT": xTb,
            "xqcols": np.ascontiguousarray(cols),
            "WqT": WqT,
            "WkT": WkT,
            "WvT": WvT,
            "mask": mask_hi if c < 4 else mask_lo,
            "ident": np.eye(128, dtype=np.float16),
        })
    return in_maps


def _host_gather(results):
    Z = np.empty((BATCH, SEQ, P), dtype=np.float32)
    for c in range(N_CORES):
        b = c % 4
        o = results[c]["out"]
        for i, t in enumerate(_tiles_for_core(c)):
            Z[b, t * QT:(t + 1) * QT, :] = o[i * QT:(i + 1) * QT, :]
    return Z


_NC_CACHE = None


def kernel(inputs, Wq, Wk, Wv):
    global _NC_CACHE
    if _NC_CACHE is None:
        _NC_CACHE = build_program()
    in_maps = _host_prep(inputs, Wq, Wk, Wv)
    res = run_bass_kernel_spmd(_NC_CACHE, in_maps, list(range(N_CORES)))
    return _host_gather(res.results)


# revision 18
# speedup vs baseline: 1.0185x; 1.0185x over previous
"""Causal self-attention (single head) on 8 Trainium2 NeuronCores.

Sharding: 8 cores = 4 batches x 2 query-tile parity sets. Core c handles
batch (c % 4). Cores 0-3 take query tiles t in {15,13,...,1} (128 rows
each), cores 4-7 take t in {14,12,...,0}. Attention iteration i=0..7 uses
a fixed causal extent E(i) = 16-2i k-tiles, so a single SPMD program
serves all cores; even-parity cores waste one fully-masked k-tile per
iteration.

Host passes x.T (plus the core's own query columns pre-gathered) and W.T
per core so the device never transposes inputs; operands are fp16 with
f32 PSUM accumulation. Softmax skips max-subtraction (scores/32 stay in a
safe exp range) and gets row sums free via the activation accum_out. All
operands stay SBUF-resident; Q.T is produced straight into SBUF.
"""

import sys

for _p in ("/opt/trn_rl_repo", "/root/.axon_site/_ro/trn_rl_repo"):
    if _p not in sys.path:
        sys.path.append(_p)

import numpy as np

import concourse.bass as bass  # noqa: F401
import concourse.mybir as mybir
import concourse.tile as tile
from concourse import bacc
from concourse.bass_utils import run_bass_kernel_spmd

F32 = mybir.dt.float32
F16 = mybir.dt.float16

BATCH, SEQ, D, P = 4, 2048, 1024, 1024
N_CORES = 8
QT = 128          # query tile rows
KTL = 128         # key tile
NBLK = 512        # matmul moving free dim
ND = D // 128     # 8 d-tiles
NP = P // 128     # 8 p-tiles
NKT = SEQ // KTL  # 16 k-tiles
NQT = 8           # q-tiles per core
SCALE = 1.0 / float(np.sqrt(P))
NEG = -1e9


def _extent(i):
    return 16 - 2 * i


def _chunks(width):
    out = []
    w = width
    while w >= NBLK:
        out.append(NBLK)
        w -= NBLK
    if w:
        assert w == 256, w
        out.append(256)
    return out


def build_program():
    nc = bacc.Bacc("TRN2", target_bir_lowering=False)

    xT = nc.dram_tensor("xT", [D, SEQ], F16, kind="ExternalInput")
    xq_in = nc.dram_tensor("xqcols", [D, NQT * QT], F16, kind="ExternalInput")
    WqT = nc.dram_tensor("WqT", [D, P], F16, kind="ExternalInput")
    WkT = nc.dram_tensor("WkT", [D, P], F16, kind="ExternalInput")
    WvT = nc.dram_tensor("WvT", [D, P], F16, kind="ExternalInput")
    mask = nc.dram_tensor("mask", [QT, 256], F32, kind="ExternalInput")
    ident_in = nc.dram_tensor("ident", [128, 128], F16, kind="ExternalInput")
    out = nc.dram_tensor("out", [NQT * QT, P], F32, kind="ExternalOutput")

    # [128, dt, cols] views (partition dim first); full-row reads keep the
    # DMA's contiguous runs at row length (2-4KB), not a sliced 1KB.
    xT_r = xT.rearrange("(dt dp) s -> dp dt s", dp=128)
    xq_r = xq_in.rearrange("(dt dp) q -> dp dt q", dp=128)
    wq_r = WqT.rearrange("(dt dp) p -> dp dt p", dp=128)
    wk_r = WkT.rearrange("(dt dp) p -> dp dt p", dp=128)
    wv_r = WvT.rearrange("(dt dp) p -> dp dt p", dp=128)

    with tile.TileContext(nc) as tc:
        with (
            tc.tile_pool(name="resident", bufs=1) as resident,
            tc.tile_pool(name="wrow", bufs=2) as wrow,
            tc.tile_pool(name="small", bufs=6) as small,
            tc.tile_pool(name="outp", bufs=2) as outp,
            tc.tile_pool(name="xqp", bufs=1) as xqp,
            tc.tile_pool(name="p0psum", bufs=2, space="PSUM") as p0psum,
            tc.tile_pool(name="spsum", bufs=2, space="PSUM") as spsum,
            tc.tile_pool(name="zpsum", bufs=2, space="PSUM") as zpsum,
            tc.tile_pool(name="tpsum", bufs=2, space="PSUM") as tpsum,
        ):
            kt_sb = resident.tile([128, NP, SEQ], F16)    # K.T  [p, k]
            v_sb = resident.tile([128, NKT, P], F16)      # V    [k, p]
            qt_all = resident.tile([128, NP, NQT * QT], F16)  # Q.T [p, q]
            xk_all = resident.tile([128, ND, SEQ], F16)   # x.T resident
            wq_sb = resident.tile([128, ND, P], F16)
            wk_sb = resident.tile([128, ND, P], F16)
            wv_sb = resident.tile([128, ND, P], F16)
            mask_sb = resident.tile([QT, 256], F32)
            ident = resident.tile([128, 128], F16)

            # startup loads: small constants, then per-d wq/xq pieces on the
            # sync queue (compute starts after the first pieces); bulk
            # tensors on the scalar queue, needed only ~40us in.
            nc.sync.dma_start(out=mask_sb, in_=mask[:, :])
            nc.sync.dma_start(out=ident, in_=ident_in[:, :])
            xq0 = xqp.tile([128, ND, NBLK], F16, tag="xq")
            for d in range(ND):
                nc.sync.dma_start(out=wq_sb[:, d, :], in_=wq_r[:, d, :])
                nc.sync.dma_start(out=xq0[:, d, :], in_=xq_r[:, d, 0:NBLK])
            nc.scalar.dma_start(out=xk_all, in_=xT_r)
            nc.scalar.dma_start(out=wk_sb, in_=wk_r)
            nc.scalar.dma_start(out=wv_sb, in_=wv_r)

            # --- Q.T production straight into SBUF ---
            for qg in range(2):
                if qg == 0:
                    xq = xq0
                else:
                    xq = xqp.tile([128, ND, NBLK], F16, tag="xq")
                    nc.sync.dma_start(
                        out=xq, in_=xq_r[:, :, qg * NBLK:(qg + 1) * NBLK])
                for pt in range(NP):
                    ps = p0psum.tile([128, NBLK], F32, tag="p0")
                    for d in range(ND):
                        nc.tensor.matmul(
                            ps,
                            wq_sb[:, d, pt * 128:(pt + 1) * 128],
                            xq[:, d, :],
                            start=(d == 0),
                            stop=(d == ND - 1),
                        )
                    nc.scalar.copy(
                        qt_all[:, pt, qg * NBLK:(qg + 1) * NBLK], ps)

            # --- K.T and V production (no DMA on the critical path) ---
            for kb in range(SEQ // NBLK):
                for pt in range(NP):
                    ps = p0psum.tile([128, NBLK], F32, tag="p0")
                    for d in range(ND):
                        nc.tensor.matmul(
                            ps,
                            wk_sb[:, d, pt * 128:(pt + 1) * 128],
                            xk_all[:, d, kb * NBLK:(kb + 1) * NBLK],
                            start=(d == 0),
                            stop=(d == ND - 1),
                        )
                    nc.scalar.copy(kt_sb[:, pt, kb * NBLK:(kb + 1) * NBLK], ps)
                for j in range(NBLK // KTL):
                    ktile = kb * (NBLK // KTL) + j
                    for pb in range(2):
                        ps = p0psum.tile([128, NBLK], F32, tag="p0")
                        for d in range(ND):
                            nc.tensor.matmul(
                                ps,
                                xk_all[:, d, ktile * 128:(ktile + 1) * 128],
                                wv_sb[:, d, pb * NBLK:(pb + 1) * NBLK],
                                start=(d == 0),
                                stop=(d == ND - 1),
                            )
                        nc.vector.tensor_copy(
                            v_sb[:, ktile, pb * NBLK:(pb + 1) * NBLK], ps)

            # --- attention, smallest extent first (unlocks earliest) ---
            for i in (7, 6, 5, 4, 3, 2, 1, 0):
                ext = _extent(i)
                width = ext * KTL
                chunks = _chunks(width)

                s_ps = []
                off = 0
                for cw in chunks:
                    ps_full = spsum.tile([QT, NBLK], F32, tag="s")
                    ps = ps_full[:, :cw]
                    for pt in range(NP):
                        nc.tensor.matmul(
                            ps,
                            qt_all[:, pt, i * QT:(i + 1) * QT],
                            kt_sb[:, pt, off:off + cw],
                            start=(pt == 0),
                            stop=(pt == NP - 1),
                        )
                    s_ps.append((ps, off, cw))
                    off += cw

                # additive causal mask on the last 256 columns of the row
                last_ps, _, last_w = s_ps[-1]
                nc.vector.tensor_add(
                    last_ps[:, last_w - 256:last_w],
                    last_ps[:, last_w - 256:last_w],
                    mask_sb,
                )

                # exp((s + m) * scale) -> fp16 weights row; row sums free
                w_sb = wrow.tile([QT, width], F16, tag="w")
                lparts = small.tile([QT, len(chunks)], F32, tag="lp")
                for ci, (ps, off_c, cw) in enumerate(s_ps):
                    nc.scalar.activation(
                        w_sb[:, off_c:off_c + cw],
                        ps,
                        mybir.ActivationFunctionType.Exp,
                        scale=SCALE,
                        accum_out=lparts[:, ci:ci + 1],
                    )

                lsum = small.tile([QT, 1], F32, tag="ls")
                nc.vector.reduce_sum(lsum, lparts, axis=mybir.AxisListType.X)
                rl = small.tile([QT, 1], F32, tag="rl")
                nc.vector.reciprocal(rl, lsum)

                # AV: transpose each weight block on PE, accumulate Z
                z0 = zpsum.tile([QT, NBLK], F32, tag="z")
                z1 = zpsum.tile([QT, NBLK], F32, tag="z")
                for kt in range(ext):
                    tp = tpsum.tile([128, 128], F16, tag="tp")
                    nc.tensor.transpose(
                        tp, w_sb[:, kt * 128:(kt + 1) * 128], ident)
                    wT = small.tile([128, 128], F16, tag="wT")
                    nc.vector.tensor_copy(wT, tp)
                    nc.tensor.matmul(
                        z0, wT, v_sb[:, kt, 0:NBLK],
                        start=(kt == 0), stop=(kt == ext - 1),
                    )
                    nc.tensor.matmul(
                        z1, wT, v_sb[:, kt, NBLK:P],
                        start=(kt == 0), stop=(kt == ext - 1),
                    )

                o_sb = outp.tile([QT, P], F32, tag="o")
                nc.vector.tensor_scalar_mul(o_sb[:, 0:NBLK], z0, rl)
                nc.vector.tensor_scalar_mul(o_sb[:, NBLK:P], z1, rl)
                nc.sync.dma_start(out=out[i * QT:(i + 1) * QT, :], in_=o_sb)

    nc.compile()
    return nc


def _tiles_for_core(c):
    """Global 128-row query-tile indices, in program order i=0..7."""
    return [(15 - 2 * i) if c < 4 else (14 - 2 * i) for i in range(NQT)]


def _host_prep(inputs, Wq, Wk, Wv):
    x = np.asarray(inputs, dtype=np.float32)
    WqT = np.ascontiguousarray(
        np.asarray(Wq, dtype=np.float32).T.astype(np.float16))
    WkT = np.ascontiguousarray(
        np.asarray(Wk, dtype=np.float32).T.astype(np.float16))
    WvT = np.ascontiguousarray(
        np.asarray(Wv, dtype=np.float32).T.astype(np.float16))

    qi = np.arange(QT)[:, None]
    ki = np.arange(128)[None, :]
    tri = np.where(qi >= ki, 0.0, NEG).astype(np.float32)
    mask_hi = np.concatenate([np.zeros((QT, 128), np.float32), tri], axis=1)
    mask_lo = np.concatenate(
        [tri, np.full((QT, 128), NEG, np.float32)], axis=1)

    in_maps = []
    xT_cache = {}
    for c in range(N_CORES):
        b = c % 4
        if b not in xT_cache:
            xT_cache[b] = np.ascontiguousarray(x[b].T.astype(np.float16))
        xTb = xT_cache[b]
        cols = np.concatenate(
            [xTb[:, t * QT:(t + 1) * QT] for t in _tiles_for_core(c)], axis=1)
        in_maps.append({
            "xT": xTb,
            "xqcols": np.ascontiguousarray(cols),
            "WqT": WqT,
            "WkT": WkT,
            "WvT": WvT,
            "mask": mask_hi if c < 4 else mask_lo,
            "ident": np.eye(128, dtype=np.float16),
        })
    return in_maps


def _host_gather(results):
    Z = np.empty((BATCH, SEQ, P), dtype=np.float32)
    for c in range(N_CORES):
        b = c % 4
        o = results[c]["out"]
        for i, t in enumerate(_tiles_for_core(c)):
            Z[b, t * QT:(t + 1) * QT, :] = o[i * QT:(i + 1) * QT, :]
    return Z


_NC_CACHE = None


def kernel(inputs, Wq, Wk, Wv):
    global _NC_CACHE
    if _NC_CACHE is None:
        _NC_CACHE = build_program()
    in_maps = _host_prep(inputs, Wq, Wk, Wv)
    res = run_bass_kernel_spmd(_NC_CACHE, in_maps, list(range(N_CORES)))
    return _host_gather(res.results)


# revision 37
# speedup vs baseline: 1.3107x; 1.2869x over previous
"""Causal self-attention (single head) on 8 Trainium2 NeuronCores.

Sharding: 8 cores = 4 batches x 2 query-tile parity sets. Core c handles
batch (c % 4). Cores 0-3 take query tiles t in {15,13,...,1} (128 rows
each), cores 4-7 take t in {14,12,...,0}. Attention iteration i=0..7 uses
a fixed causal extent E(i) = 16-2i k-tiles, so a single SPMD program
serves all cores; even-parity cores waste one fully-masked k-tile per
iteration.

Host passes x.T (plus the core's own query columns pre-gathered) and W.T
per core so the device never transposes inputs; operands are fp16 with
f32 PSUM accumulation. Softmax skips max-subtraction (scores/32 stay in a
safe exp range) and gets row sums free via the activation accum_out. All
operands stay SBUF-resident; Q.T is produced straight into SBUF.
"""

import sys

for _p in ("/opt/trn_rl_repo", "/root/.axon_site/_ro/trn_rl_repo"):
    if _p not in sys.path:
        sys.path.append(_p)

import numpy as np

import concourse.bass as bass  # noqa: F401
import concourse.mybir as mybir
import concourse.tile as tile
from concourse import bacc
from concourse.bass_utils import run_bass_kernel_spmd

F32 = mybir.dt.float32
F16 = mybir.dt.float16

BATCH, SEQ, D, P = 4, 2048, 1024, 1024
N_CORES = 8
QT = 128          # query tile rows
KTL = 128         # key tile
NBLK = 512        # matmul moving free dim
ND = D // 128     # 8 d-tiles
NP = P // 128     # 8 p-tiles
NKT = SEQ // KTL  # 16 k-tiles
NQT = 8           # q-tiles per core
SCALE = 1.0 / float(np.sqrt(P))
NEG = -1e9


def _extent(i):
    return 16 - 2 * i


def _chunks(width):
    out = []
    w = width
    while w >= NBLK:
        out.append(NBLK)
        w -= NBLK
    if w:
        assert w == 256, w
        out.append(256)
    return out


def build_program():
    nc = bacc.Bacc("TRN2", target_bir_lowering=False)

    xT = nc.dram_tensor("xT", [D, SEQ], F16, kind="ExternalInput")
    xn = nc.dram_tensor("xn", [SEQ, D], F16, kind="ExternalInput")
    xq_in = nc.dram_tensor("xqcols", [D, NQT * QT], F16, kind="ExternalInput")
    AT = nc.dram_tensor("AT", [D, D], F16, kind="ExternalInput")
    WvT = nc.dram_tensor("WvT", [D, P], F16, kind="ExternalInput")
    mask = nc.dram_tensor("mask", [QT, 256], F32, kind="ExternalInput")
    ident_in = nc.dram_tensor("ident", [128, 128], F16, kind="ExternalInput")
    out = nc.dram_tensor("out", [NQT * QT, P], F32, kind="ExternalOutput")

    # [128, dt, cols] views (partition dim first); full-row reads keep the
    # DMA's contiguous runs at row length (2-4KB), not a sliced 1KB.
    xT_r = xT.rearrange("(dt dp) s -> dp dt s", dp=128)
    xn_r = xn.rearrange("(kt kp) d -> kp kt d", kp=128)
    xq_r = xq_in.rearrange("(dt dp) q -> dp dt q", dp=128)
    at_r = AT.rearrange("(dt dp) d -> dp dt d", dp=128)
    wv_r = WvT.rearrange("(dt dp) p -> dp dt p", dp=128)

    with tile.TileContext(nc) as tc:
        with (
            tc.tile_pool(name="resident", bufs=1) as resident,
            tc.tile_pool(name="wrow", bufs=2) as wrow,
            tc.tile_pool(name="small", bufs=6) as small,
            tc.tile_pool(name="outp", bufs=2) as outp,
            tc.tile_pool(name="p0psum", bufs=3, space="PSUM") as p0psum,
            tc.tile_pool(name="zpsum", bufs=3, space="PSUM") as zpsum,
            tc.tile_pool(name="tpsum", bufs=2, space="PSUM") as tpsum,
        ):
            kt_sb = resident.tile([128, NP, SEQ], F16)    # G = A x^T [d, k]
            xn_all = resident.tile([128, NKT, D], F16)    # x natural [k, d]
            xq_all = resident.tile([128, ND, NQT * QT], F16)  # x.T q-cols
            xk_all = resident.tile([128, ND, SEQ], F16)   # x.T resident
            at_sb = resident.tile([128, ND, D], F16)      # A^T = Wk^T Wq
            wv_sb = resident.tile([128, ND, P], F16)
            mask_sb = resident.tile([QT, 256], F32)
            ident = resident.tile([128, 128], F16)
            cbias = resident.tile([QT, 1], F32)
            nc.vector.memset(cbias, -4.0)

            # startup loads: small constants, then per-d wq/xq pieces on the
            # sync queue (compute starts after the first pieces); bulk
            # tensors on the scalar queue, needed only ~40us in.
            # G-loop inputs by d-tile on the sync queue (full rows keep
            # 2-4KB DMA runs); everything else on the scalar queue.
            nc.sync.dma_start(out=ident, in_=ident_in[:, :])
            H = SEQ // 2
            for d in range(ND):
                nc.sync.dma_start(out=at_sb[:, d, :], in_=at_r[:, d, :])
                nc.sync.dma_start(
                    out=xk_all[:, d, 0:H], in_=xT_r[:, d, 0:H])
            for d in range(ND):
                nc.sync.dma_start(
                    out=xk_all[:, d, H:SEQ], in_=xT_r[:, d, H:SEQ])
            nc.scalar.dma_start(out=mask_sb, in_=mask[:, :])
            nc.scalar.dma_start(out=xq_all, in_=xq_r)
            nc.scalar.dma_start(
                out=xn_all[:, 0:NKT // 2, :], in_=xn_r[:, 0:NKT // 2, :])
            nc.scalar.dma_start(out=wv_sb, in_=wv_r)
            nc.scalar.dma_start(
                out=xn_all[:, NKT // 2:NKT, :], in_=xn_r[:, NKT // 2:NKT, :])

            # --- G = A x^T and V production ---
            for kb in range(SEQ // NBLK):
                for pt in range(NP):
                    ps = p0psum.tile([128, NBLK], F32, tag="p0")
                    for d in range(ND):
                        nc.tensor.matmul(
                            ps,
                            at_sb[:, d, pt * 128:(pt + 1) * 128],
                            xk_all[:, d, kb * NBLK:(kb + 1) * NBLK],
                            start=(d == 0),
                            stop=(d == ND - 1),
                        )
                    nc.scalar.copy(kt_sb[:, pt, kb * NBLK:(kb + 1) * NBLK], ps)

            # --- attention, smallest extent first (unlocks earliest) ---
            for i in (7, 6, 5, 4, 3, 2, 1, 0):
                ext = _extent(i)
                width = ext * KTL
                chunks = _chunks(width)

                s_ps = []
                off = 0
                for cw in chunks:
                    ps_full = p0psum.tile([QT, NBLK], F32, tag="p0")
                    ps = ps_full[:, :cw]
                    for pt in range(NP):
                        nc.tensor.matmul(
                            ps,
                            xq_all[:, pt, i * QT:(i + 1) * QT],
                            kt_sb[:, pt, off:off + cw],
                            start=(pt == 0),
                            stop=(pt == NP - 1),
                        )
                    s_ps.append((ps, off, cw))
                    off += cw

                # additive causal mask on the last 256 columns of the row
                last_ps, _, last_w = s_ps[-1]
                nc.vector.tensor_add(
                    last_ps[:, last_w - 256:last_w],
                    last_ps[:, last_w - 256:last_w],
                    mask_sb,
                )

                # exp((s + m) * scale) -> fp16 weights row; row sums free
                w_sb = wrow.tile([QT, width], F16, tag="w")
                lparts = small.tile([QT, len(chunks)], F32, tag="lp")
                for ci, (ps, off_c, cw) in enumerate(s_ps):
                    nc.scalar.activation(
                        w_sb[:, off_c:off_c + cw],
                        ps,
                        mybir.ActivationFunctionType.Exp,
                        scale=SCALE,
                        bias=cbias,
                        accum_out=lparts[:, ci:ci + 1],
                    )

                lsum = small.tile([QT, 1], F32, tag="ls")
                nc.vector.reduce_sum(lsum, lparts, axis=mybir.AxisListType.X)
                rl = small.tile([QT, 1], F32, tag="rl")
                nc.vector.reciprocal(rl, lsum)

                # U = W x  (transpose each weight block on PE)
                u0 = zpsum.tile([QT, NBLK], F32, tag="z")
                u1 = zpsum.tile([QT, NBLK], F32, tag="z")
                for kt in range(ext):
                    tp = tpsum.tile([128, 128], F16, tag="tp")
                    nc.tensor.transpose(
                        tp, w_sb[:, kt * 128:(kt + 1) * 128], ident)
                    wT = small.tile([128, 128], F16, tag="wT")
                    nc.vector.tensor_copy(wT, tp)
                    nc.tensor.matmul(
                        u0, wT, xn_all[:, kt, 0:NBLK],
                        start=(kt == 0), stop=(kt == ext - 1),
                    )
                    nc.tensor.matmul(
                        u1, wT, xn_all[:, kt, NBLK:D],
                        start=(kt == 0), stop=(kt == ext - 1),
                    )
                u_sb = wrow.tile([QT, D], F16, tag="u")
                nc.scalar.copy(u_sb[:, 0:NBLK], u0)
                nc.vector.tensor_copy(u_sb[:, NBLK:D], u1)

                # Z = U Wv^T  (U transposed per d-tile on PE)
                uT_sb = small.tile([128, ND, 128], F16, tag="uT")
                for dt in range(ND):
                    tpu = tpsum.tile([128, 128], F16, tag="tp")
                    nc.tensor.transpose(
                        tpu, u_sb[:, dt * 128:(dt + 1) * 128], ident)
                    nc.vector.tensor_copy(uT_sb[:, dt, :], tpu)
                z0 = zpsum.tile([QT, NBLK], F32, tag="z")
                z1 = zpsum.tile([QT, NBLK], F32, tag="z")
                for dt in range(ND):
                    nc.tensor.matmul(
                        z0, uT_sb[:, dt, :], wv_sb[:, dt, 0:NBLK],
                        start=(dt == 0), stop=(dt == ND - 1),
                    )
                    nc.tensor.matmul(
                        z1, uT_sb[:, dt, :], wv_sb[:, dt, NBLK:P],
                        start=(dt == 0), stop=(dt == ND - 1),
                    )

                o_sb = outp.tile([QT, P], F32, tag="o")
                nc.vector.tensor_scalar_mul(o_sb[:, 0:NBLK], z0, rl)
                nc.vector.tensor_scalar_mul(o_sb[:, NBLK:P], z1, rl)
                nc.sync.dma_start(out=out[i * QT:(i + 1) * QT, :], in_=o_sb)

    nc.compile()
    return nc


def _tiles_for_core(c):
    """Global 128-row query-tile indices, in program order i=0..7."""
    return [(15 - 2 * i) if c < 4 else (14 - 2 * i) for i in range(NQT)]


def _host_prep(inputs, Wq, Wk, Wv):
    x = np.asarray(inputs, dtype=np.float32)
    Wqf = np.asarray(Wq, dtype=np.float32)
    Wkf = np.asarray(Wk, dtype=np.float32)
    # scores = x (Wq^T Wk) x^T; device stationary wants the transpose
    ATm = np.ascontiguousarray((Wkf.T @ Wqf).astype(np.float16))
    WvT = np.ascontiguousarray(
        np.asarray(Wv, dtype=np.float32).T.astype(np.float16))

    qi = np.arange(QT)[:, None]
    ki = np.arange(128)[None, :]
    tri = np.where(qi >= ki, 0.0, NEG).astype(np.float32)
    mask_hi = np.concatenate([np.zeros((QT, 128), np.float32), tri], axis=1)
    mask_lo = np.concatenate(
        [tri, np.full((QT, 128), NEG, np.float32)], axis=1)

    in_maps = []
    xT_cache = {}
    for c in range(N_CORES):
        b = c % 4
        if b not in xT_cache:
            xT_cache[b] = np.ascontiguousarray(x[b].T.astype(np.float16))
        xTb = xT_cache[b]
        cols = np.concatenate(
            [xTb[:, t * QT:(t + 1) * QT] for t in _tiles_for_core(c)], axis=1)
        in_maps.append({
            "xT": xTb,
            "xn": np.ascontiguousarray(x[b].astype(np.float16)),
            "xqcols": np.ascontiguousarray(cols),
            "AT": ATm,
            "WvT": WvT,
            "mask": mask_hi if c < 4 else mask_lo,
            "ident": np.eye(128, dtype=np.float16),
        })
    return in_maps


def _host_gather(results):
    Z = np.empty((BATCH, SEQ, P), dtype=np.float32)
    for c in range(N_CORES):
        b = c % 4
        o = results[c]["out"]
        for i, t in enumerate(_tiles_for_core(c)):
            Z[b, t * QT:(t + 1) * QT, :] = o[i * QT:(i + 1) * QT, :]
    return Z


_NC_CACHE = None


def kernel(inputs, Wq, Wk, Wv):
    global _NC_CACHE
    if _NC_CACHE is None:
        _NC_CACHE = build_program()
    in_maps = _host_prep(inputs, Wq, Wk, Wv)
    # The first execution after a fresh compile occasionally hits a
    # transient NRT_EXEC_UNIT_UNRECOVERABLE; a retry reliably succeeds.
    last_err = None
    Z = None
    for _ in range(3):
        try:
            res = run_bass_kernel_spmd(
                _NC_CACHE, in_maps, list(range(N_CORES)))
            Z = _host_gather(res.results)
            if np.isfinite(Z).all():
                return Z
        except Exception as e:  # noqa: BLE001
            last_err = e
    if Z is not None:
        return Z
    raise last_err
